# revision 29
# baseline (speedup 1.0000x reference)
"""Trainium2 Bass kernel for nn_AutoDecoder (8-layer MLP trunk + CP triple-product einsum).

Sharding: feature-parallel over the einsum feature dim FD=32 -> 4 features per core.
Each core runs the full trunk for all 64 batches (small), a 3840-row slice of the
30720-row final layer, and the einsum for its 4 features over all batches.
Outputs are disjoint slices of grid[:, f, :, :, :]; no collectives needed.

Precision: trunk matmuls in float32r (full-rate PE streaming, ~TF32 precision),
LayerNorm math in f32, layer 8 + einsum matmuls in bf16 with f32 PSUM
accumulation, output stored bf16 and upcast host-side (end-to-end rel err
~8e-3 vs the f32 reference, well under the 2e-2 gate).

Layout tricks (all weight reshuffles done host-side in numpy):
 - trunk weights pre-transposed [in, out] so they feed matmul rhs directly;
   z passed both ways (zT for layer-1 lhsT, zb for the layer-4 concat)
 - w8 shard rows permuted to g = ((s*4+fl)*10+r)*32+i; layer 8 computed
   directly in out8T [row, b] orientation, bounced to DRAM per s-section and
   gathered into X/Y/Z operand tensors [128 part=(fl,r)+pad, (coord,b) free]
   (DRAM round-trip because SBUF partition starts must be quadrant-aligned)
 - einsum per batch b: block-diagonal lhsT [128, 128] (4 features x 10 ranks,
   zero-padded), P = y (x) z outer product [128, 1024] built in one DVE/GpSimd
   op via step-0 broadcast APs, two bf16 matmuls -> PSUM [128, 1024] f32 ->
   bf16 SBUF -> one 256KB HBM write per batch
 - per-layer bias/gamma/beta broadcast from a packed [1, *] vector via
   step-0-partition DMA src APs; LayerNorm fused with scalar_tensor_tensor
   accum_out (bias-add+rowsum, square+rowsum in single DVE ops)
Engine assignment spreads the einsum phase across DVE/GpSimd (P outer
products), ACT+DVE (PSUM->SBUF casts), both HWDGE queues (loads vs stores).
"""

import sys

sys.path.insert(0, "/opt/trn_rl_repo")

import numpy as np

import concourse.bass as bass
import concourse.mybir as mybir
from concourse import bacc, tile
from concourse.bass_utils import run_bass_kernel_spmd

F32 = mybir.dt.float32
F32R = mybir.dt.float32r
BF16 = mybir.dt.bfloat16

B = 64
LATENT = 256
HIDDEN = 512
NUM_LAYERS = 8
S, RC, V, FD = 3, 10, 32, 32
EPS = 1e-5
N_CORES = 8
FL = FD // N_CORES          # 4 features per core
NROW = S * RC * V * FL      # 3840 rows of w8 per core
NTILE = NROW // 128         # 30 partition tiles of out8T
GRID_PER_B = FL * V * V * V  # 131072 output elems per batch per core

IN_DIMS = {i: (LATENT if i == 1 else HIDDEN) for i in range(1, NUM_LAYERS)}
OUT_DIMS = {i: (HIDDEN - LATENT if i == 4 else HIDDEN) for i in range(1, NUM_LAYERS)}

# packed per-layer bias/gamma/beta vector offsets inside "gbb"
_GBB_OFFSETS = {}
_off = 0
for _i in range(1, NUM_LAYERS):
    _GBB_OFFSETS[("b", _i)] = _off
    _off += OUT_DIMS[_i]
    _GBB_OFFSETS[("g", _i)] = _off
    _off += HIDDEN
    _GBB_OFFSETS[("beta", _i)] = _off
    _off += HIDDEN
GBB_LEN = _off

NGRP = 16  # einsum batches per lhsT round


def build_nc(loop_n=None):
    """loop_n: wrap the whole kernel body in a hardware For_i loop (used only
    for repeat-timing on hardware; the graded kernel uses loop_n=None)."""
    nc = bacc.Bacc("TRN2", target_bir_lowering=False, debug=False,
                   num_devices=N_CORES)

    zT = nc.declare_dram_parameter("zT", [LATENT, B], F32R, isOutput=False)
    zb = nc.declare_dram_parameter("zb", [B, LATENT], F32, isOutput=False)
    wT = {
        i: nc.declare_dram_parameter(f"w{i}T", [IN_DIMS[i], OUT_DIMS[i]], F32R, isOutput=False)
        for i in range(1, NUM_LAYERS)
    }
    gbb = nc.declare_dram_parameter("gbb", [1, GBB_LEN], F32, isOutput=False)
    w8T = nc.declare_dram_parameter("w8T", [HIDDEN, NROW], BF16, isOutput=False)
    b8c = nc.declare_dram_parameter("b8c", [128, NTILE], F32, isOutput=False)
    ident = nc.declare_dram_parameter("ident64", [B, B], F32, isOutput=False)
    out_ext = nc.declare_dram_parameter("out", [B, GRID_PER_B], BF16, isOutput=True)

    with tile.TileContext(nc) as tc:
        if loop_n is None:
            _build_graph(nc, tc, zT, zb, wT, gbb, w8T, b8c, ident, out_ext)
        else:
            with tc.For_i(0, loop_n, 1):
                _build_graph(nc, tc, zT, zb, wT, gbb, w8T, b8c, ident, out_ext)
    if not nc.is_finalized():
        nc.finalize()
    return nc


def _build_graph(nc, tc, zT, zb, wT, gbb, w8T, b8c, ident, out_ext):
    from contextlib import ExitStack

    with ExitStack() as ctx:
        consts = ctx.enter_context(tc.tile_pool(name="consts", bufs=1))
        scr = ctx.enter_context(tc.tile_pool(name="scr", bufs=4))       # bcast bias/g/beta
        xwork = ctx.enter_context(tc.tile_pool(name="xwork", bufs=2))   # trunk stage tiles
        stats = ctx.enter_context(tc.tile_pool(name="stats", bufs=2))   # LN scalars
        xtp = ctx.enter_context(tc.tile_pool(name="xtp", bufs=8))       # transposed x chunks
        w8pool = ctx.enter_context(tc.tile_pool(name="w8pool", bufs=6))
        ppool = ctx.enter_context(tc.tile_pool(name="ppool", bufs=4))
        gpool = ctx.enter_context(tc.tile_pool(name="gpool", bufs=4))
        ps_small = ctx.enter_context(tc.tile_pool(name="ps_small", bufs=2, space="PSUM"))
        ps_grid = ctx.enter_context(tc.tile_pool(name="ps_grid", bufs=3, space="PSUM"))
        dram = ctx.enter_context(tc.tile_pool(name="dram", bufs=1, space="DRAM"))

        # ---- resident constants (loaded on the SP HWDGE queue: SP is idle
        # during the trunk, and the big output writes only start later) ----
        id_sb = consts.tile([B, B], F32, tag="ident")
        nc.sync.dma_start(id_sb[:], ident[:])
        zT_sb = []
        for kc in range(LATENT // 128):
            t = consts.tile([128, B], F32R, tag=f"zT{kc}")
            nc.sync.dma_start(t[:], zT[kc * 128:(kc + 1) * 128, :])
            zT_sb.append(t)
        zb_sb = consts.tile([B, LATENT], F32, tag="zb")
        nc.sync.dma_start(zb_sb[:], zb[:])
        w_sb = {}
        for i in range(1, NUM_LAYERS):
            for kc in range(IN_DIMS[i] // 128):
                t = consts.tile([128, OUT_DIMS[i]], F32R, tag=f"w{i}_{kc}")
                nc.sync.dma_start(t[:], wT[i][kc * 128:(kc + 1) * 128, :])
                w_sb[(i, kc)] = t
        b8_sb = consts.tile([128, NTILE], F32, tag="b8")
        nc.sync.dma_start(b8_sb[:], b8c[:])

        def bcast_param(kind, i, n):
            """[1, n] slice of gbb broadcast to [B, n] in SBUF via step-0 DMA src AP."""
            off = _GBB_OFFSETS[(kind, i)]
            t = scr.tile([B, n], F32, tag="bcast")
            src = gbb[0:1, off:off + n].partition_broadcast(B)
            nc.sync.dma_start(t[:], src)
            return t

        # ---- MLP trunk, layers 1..7; x kept as [B, feat], lhsT chunks [128, B] ----
        cur_lhsT = zT_sb
        for i in range(1, NUM_LAYERS):
            ind, outd = IN_DIMS[i], OUT_DIMS[i]
            ps = ps_small.tile([B, outd], F32, tag="mm")
            nkc = ind // 128
            for kc in range(nkc):
                nc.tensor.matmul(
                    ps[:], cur_lhsT[kc][:], w_sb[(i, kc)][:],
                    start=(kc == 0), stop=(kc == nkc - 1),
                )
            bias_t = bcast_param("b", i, outd)
            xb = xwork.tile([B, HIDDEN], F32, tag="xb")
            s1 = stats.tile([B, 1], F32, tag="s1")
            if i == 4:
                nc.vector.tensor_tensor(xb[:, :outd], ps[:], bias_t[:],
                                        op=mybir.AluOpType.add)
                nc.vector.tensor_copy(xb[:, outd:], zb_sb[:])
                nc.vector.reduce_sum(s1[:], xb[:], axis=mybir.AxisListType.X)
            else:
                # bias-add + row-sum in one op
                nc.vector.scalar_tensor_tensor(xb[:], ps[:], 1.0, bias_t[:],
                                               op0=mybir.AluOpType.mult,
                                               op1=mybir.AluOpType.add,
                                               accum_out=s1[:])
            mu = stats.tile([B, 1], F32, tag="mu")
            nc.vector.tensor_scalar_mul(mu[:], s1[:], 1.0 / HIDDEN)
            xc = xwork.tile([B, HIDDEN], F32, tag="xc")
            nc.vector.tensor_scalar(xc[:], xb[:], mu[:], None, op0=mybir.AluOpType.subtract)
            # square + row-sum in one op
            sq = xwork.tile([B, HIDDEN], F32, tag="sq")
            s2 = stats.tile([B, 1], F32, tag="s2")
            nc.vector.scalar_tensor_tensor(sq[:], xc[:], 1.0, xc[:],
                                           op0=mybir.AluOpType.mult,
                                           op1=mybir.AluOpType.mult,
                                           accum_out=s2[:])
            var = stats.tile([B, 1], F32, tag="var")
            nc.vector.tensor_scalar(var[:], s2[:], 1.0 / HIDDEN, EPS,
                                    op0=mybir.AluOpType.mult, op1=mybir.AluOpType.add)
            std = stats.tile([B, 1], F32, tag="std")
            nc.scalar.activation(std[:], var[:], mybir.ActivationFunctionType.Sqrt)
            rstd = stats.tile([B, 1], F32, tag="rstd")
            nc.vector.reciprocal(rstd[:], std[:])
            g_t = bcast_param("g", i, HIDDEN)
            beta_t = bcast_param("beta", i, HIDDEN)
            y = xwork.tile([B, HIDDEN], F32, tag="y")
            nc.vector.scalar_tensor_tensor(y[:], xc[:], rstd[:], g_t[:],
                                           op0=mybir.AluOpType.mult,
                                           op1=mybir.AluOpType.mult)
            yb = xwork.tile([B, HIDDEN], F32, tag="yb")
            nc.vector.tensor_tensor(yb[:], y[:], beta_t[:], op=mybir.AluOpType.add)
            xout = xwork.tile([B, HIDDEN], F32, tag="xout")
            nc.vector.tensor_scalar_max(xout[:], yb[:], 0.0)
            # transpose for the next matmul's lhsT (bf16 after layer 7: layer 8
            # runs its matmuls in bf16)
            xt_dt = BF16 if i == NUM_LAYERS - 1 else F32R
            nxt = []
            for c in range(HIDDEN // 128):
                pt = ps_small.tile([128, B], F32, tag="mm")
                nc.tensor.transpose(pt[:], xout[:, c * 128:(c + 1) * 128], id_sb[:])
                xt = xtp.tile([128, B], xt_dt, tag=f"xt{c}")
                if c % 2 == 0:
                    nc.scalar.copy(xt[:], pt[:])
                else:
                    nc.vector.tensor_copy(xt[:], pt[:])
                nxt.append(xt)
            cur_lhsT = nxt

        # ---- layer 8: three per-s out8T sections [128, 10*B], rows in permuted
        # order g; each section is bounced to DRAM and gathered into its G
        # operand as soon as its 10 tiles finish, overlapping the rest of L8 ----
        W8COL = [0, 1024, 2048, 3072, NROW]  # col chunks of w8T (8 t-tiles each)
        w8_sb = {}

        def w8chunk(kc, tc):
            key = (kc, tc)
            if key not in w8_sb:
                c0, c1 = W8COL[tc], W8COL[tc + 1]
                t = w8pool.tile([128, c1 - c0], BF16, tag="w8chunk")
                nc.scalar.dma_start(t[:], w8T[kc * 128:(kc + 1) * 128, c0:c1])
                w8_sb[key] = t
            return w8_sb[key]

        o8d = dram.tile([NROW, B], F32, tag="o8d")
        XG = consts.tile([128, V * B], F32, tag="XG")
        YG = consts.tile([128, V * B], F32, tag="YG")
        ZG = consts.tile([128, V * B], F32, tag="ZG")
        GS = {0: XG, 1: YG, 2: ZG}
        for G in (XG, YG, ZG):
            nc.gpsimd.memset(G[:], 0.0)

        for s in range(S):
            out8s = consts.tile([128, 10 * B], F32, tag=f"out8s{s}")
            for tl in range(10):
                t = s * 10 + tl
                tcn = t // 8
                off = (t % 8) * 128
                ps = ps_small.tile([128, B], F32, tag="mm")
                for kc in range(4):
                    nc.tensor.matmul(
                        ps[:], w8chunk(kc, tcn)[:, off:off + 128], cur_lhsT[kc][:],
                        start=(kc == 0), stop=(kc == 3),
                    )
                nc.vector.tensor_scalar_add(out8s[:, tl * B:(tl + 1) * B], ps[:],
                                            b8_sb[:, t:t + 1])
            nc.sync.dma_start(
                o8d[s * 1280:(s + 1) * 1280, :].rearrange("(t p) b -> p t b", p=128),
                out8s[:].rearrange("p (t b) -> p t b", b=B),
            )
            G = GS[s]
            for fl in range(4):
                base = (s * 4 + fl) * 320
                # collapsed (j b) free dim -> 8KB-contiguous descriptors
                src = o8d[base:base + 320, :].rearrange("(r j) b -> r (j b)", j=V)
                dst = G[fl * 32:fl * 32 + 10, :]
                if fl % 2 == 0:
                    nc.scalar.dma_start(dst, src)
                else:
                    nc.sync.dma_start(dst, src)

        # ---- einsum: single block-diag lhsT for all 64 batches ----
        lhsT64 = consts.tile([128, B * 128], BF16, tag="lhsT64")
        nc.gpsimd.memset(lhsT64[:], 0.0)
        for fl in range(4):
            src = XG[fl * 32:fl * 32 + 10, :].rearrange("p (i b) -> p b i", b=B)
            dst = lhsT64[fl * 32:fl * 32 + 10, :].rearrange("p (b m) -> p b m", m=128)[
                :, :, fl * 32:(fl + 1) * 32]
            nc.gpsimd.tensor_copy(dst, src)
        for b in range(B):
            P = ppool.tile([128, V * V], BF16, tag="P")
            y_ap = (YG[:].rearrange("p (j b) -> p j b", b=B)[:, :, b:b + 1]
                    .to_broadcast([128, V, V]))
            z_ap = (ZG[:].rearrange("p (k b) -> p b k", b=B)[:, b:b + 1, :]
                    .to_broadcast([128, V, V]))
            peng = nc.vector if b % 4 == 0 else nc.gpsimd
            peng.tensor_tensor(
                P[:].rearrange("p (j k) -> p j k", k=V), y_ap, z_ap,
                op=mybir.AluOpType.mult)
            gps = ps_grid.tile([128, 1024], F32, tag="gps")
            for h in range(2):
                nc.tensor.matmul(
                    gps[:, h * 512:(h + 1) * 512],
                    lhsT64[:, b * 128:(b + 1) * 128],
                    P[:, h * 512:(h + 1) * 512],
                    start=True, stop=True,
                )
            gsb = gpool.tile([128, 1024], BF16, tag="gsb")
            if b % 3 == 0:
                nc.vector.tensor_copy(gsb[:], gps[:])
            else:
                nc.scalar.copy(gsb[:], gps[:])
            nc.sync.dma_start(out_ext[b:b + 1, :], gsb[:])


_NC_CACHE = None


def _get_nc():
    global _NC_CACHE
    if _NC_CACHE is None:
        _NC_CACHE = build_nc()
    return _NC_CACHE


def _make_in_maps(z, params):
    z = np.asarray(z, dtype=np.float32)
    p = {k: np.asarray(v, dtype=np.float32) for k, v in params.items()}

    zT = np.ascontiguousarray(z.T)
    gbb = np.zeros((1, GBB_LEN), dtype=np.float32)
    for i in range(1, NUM_LAYERS):
        for kind, key, n in (("b", f"b{i}", OUT_DIMS[i]),
                             ("g", f"g{i}", HIDDEN),
                             ("beta", f"beta{i}", HIDDEN)):
            off = _GBB_OFFSETS[(kind, i)]
            gbb[0, off:off + n] = p[key]
    wTs = {f"w{i}T": np.ascontiguousarray(p[f"w{i}"].T) for i in range(1, NUM_LAYERS)}
    ident = np.eye(B, dtype=np.float32)

    # w8 row permutation: g = ((s*4+fl)*10+r)*32+i  ->  orig = s*10240+r*1024+i*32+f
    s_ = np.arange(S)[:, None, None, None]
    fl_ = np.arange(FL)[None, :, None, None]
    r_ = np.arange(RC)[None, None, :, None]
    i_ = np.arange(V)[None, None, None, :]
    w8 = p[f"w{NUM_LAYERS}"]
    b8 = p[f"b{NUM_LAYERS}"]

    in_maps = []
    for c in range(N_CORES):
        orig = (s_ * (RC * V * FD) + r_ * (V * FD) + i_ * FD + (FL * c + fl_)).reshape(-1)
        w8s = w8[orig]                     # [3840, 512]
        w8Tc = np.ascontiguousarray(w8s.T).astype(mybir.dt.np(BF16))  # [512, 3840]
        b8s = b8[orig]
        b8cc = np.ascontiguousarray(b8s.reshape(NTILE, 128).T)  # [128, 30]
        m = {"zT": zT, "zb": z, "gbb": gbb, "w8T": w8Tc, "b8c": b8cc, "ident64": ident}
        m.update(wTs)
        in_maps.append(m)
    return in_maps


def kernel(z, params):
    nc = _get_nc()
    in_maps = _make_in_maps(z, params)
    res = run_bass_kernel_spmd(nc, in_maps, core_ids=list(range(N_CORES)))
    parts = [res.results[c]["out"].astype(np.float32).reshape(B, FL, V, V, V)
             for c in range(N_CORES)]
    return np.concatenate(parts, axis=1)
